# revision 89
# baseline (speedup 1.0000x reference)
"""BiSTSSM (bidirectional Mamba-style selective scan) on 8 Trainium2 cores.

Sharding: core c = (b, k) for the scan launch (B=4 batches x K=2 directions);
core c = (b, t-half) for the merge+LayerNorm launch.

Scan layout: partitions hold (32 d-channels x 4 states) so the whole
selective-scan recurrence for 16 states runs as 4 `tensor_tensor_scan`
instructions per 32-channel block over the full T=2048 free axis:
    h[p, t] = exp(A[p] * delta[p, t]) * h[p, t-1] + (delta*u)[p, t] * B[n(p), t]
    y[d, t] = sum_n C[n, t] * h[(d, n), t]   (PE block-diagonal reduce)
"""

import numpy as np
from contextlib import ExitStack

import concourse.bass as bass
import concourse.mybir as mybir
import concourse.tile as tile
from concourse import bacc
from concourse import hw_specs as _hw_specs
from concourse.masks import make_identity
from concourse.bass_utils import run_bass_kernel_spmd

# Steer the ACT table-set chooser away from exp-only sets so Exp and Ln both
# resolve to natural_log_exp_and_others (one table load for the whole kernel
# instead of a load per Exp<->Ln transition). Ids are positional, so the
# competing sets are emptied rather than removed.
_orig_get_tables = _hw_specs.get_activation_tables


def _patched_get_tables(module_arch):
    tables = dict(_orig_get_tables(module_arch))
    for name in ("exp_and_others", "exp_and_friends"):
        if name in tables:
            tables[name] = set()
    return tables


_hw_specs.get_activation_tables = _patched_get_tables
import concourse.bacc as _bacc_mod
if hasattr(_bacc_mod, "get_activation_tables"):
    _bacc_mod.get_activation_tables = _patched_get_tables

F = mybir.dt.float32
H16 = mybir.dt.float16
OP = mybir.AluOpType
AF = mybir.ActivationFunctionType

D_INNER, J, N, R, K = 384, 5, 16, 12, 2
B, T = 4, 2048
D = D_INNER * J          # 1920 scan channels
NT = D // 128            # 15 d-tiles of 128
NS = D // 32             # 60 sub-blocks of 32 channels
NG = 4                   # n-groups of 4 states (4 x 4 = 16 states)
TH = T // 2              # merge-launch T half


def _build_scan_nc():
    nc = bacc.Bacc()
    x_d = nc.dram_tensor("x", [D, T], H16, kind="ExternalInput")
    wxT_d = nc.dram_tensor("wxT", [D, 80], H16, kind="ExternalInput")
    wdtT_d = nc.dram_tensor("wdtT", [R, D], H16, kind="ExternalInput")
    bias_d = nc.dram_tensor("bias_r", [128, NT], F, kind="ExternalInput")
    alog_d = nc.dram_tensor("alog_r", [128, NS * NG], F, kind="ExternalInput")
    s4_d = nc.dram_tensor("s4", [128, 512], H16, kind="ExternalInput")
    ys_d = nc.dram_tensor("ys", [D, T], H16, kind="ExternalOutput")

    du_drams = [nc.dram_tensor(f"du_scr{i}", [128, T], H16) for i in range(NT)]
    dl_drams = [nc.dram_tensor(f"dl_scr{i}", [128, T], H16) for i in range(NT)]
    bc_dram = nc.dram_tensor("bc_scr", [N, T], H16)
    cc_dram = nc.dram_tensor("cc_scr", [N, T], H16)

    C44 = 80  # padded: dtr rows 0:12, B rows 32:48, C rows 64:80

    with tile.TileContext(nc) as tc, ExitStack() as ctx:
        singles = ctx.enter_context(tc.tile_pool(name="singles", bufs=1))
        xpool = ctx.enter_context(tc.tile_pool(name="xpool", bufs=2))
        wpool = ctx.enter_context(tc.tile_pool(name="wpool", bufs=2))
        dpool = ctx.enter_context(tc.tile_pool(name="dpool", bufs=2))
        spool = ctx.enter_context(tc.tile_pool(name="spool", bufs=2))
        ypool = ctx.enter_context(tc.tile_pool(name="ypool", bufs=2))
        # One PSUM pool of full-T fp32 tiles (4 banks each, bufs=2 -> all 8
        # banks). Phase A works in subviews; phase C double-buffers y across
        # d-tiles so tile i+1's matmuls overlap tile i's PSUM drain.
        pall = ctx.enter_context(tc.tile_pool(name="pall", bufs=2, space="PSUM"))

        # --- constants ---
        s4_t = singles.tile([128, 512], H16)
        nc.sync.dma_start(s4_t[:], s4_d[:, :])
        bias_t = singles.tile([128, NT], F)
        nc.sync.dma_start(bias_t[:], bias_d[:, :])
        alog_t = singles.tile([128, NS * NG], F)
        nc.sync.dma_start(alog_t[:], alog_d[:, :])
        a_t = singles.tile([128, NS * NG], F)  # A = -exp(A_logs), scan layout
        nc.scalar.activation(a_t[:], alog_t[:], AF.Exp)
        nc.vector.tensor_scalar_mul(a_t[:], a_t[:], -1.0)
        wdtT_t = singles.tile([R, D], H16)
        nc.sync.dma_start(wdtT_t[:], wdtT_d[:, :])
        wts = []
        dtr_t = singles.tile([R, T], H16)
        bc_t = singles.tile([N, T], H16)
        cc_t = singles.tile([N, T], H16)
        neg87_t = singles.tile([128, 1], F)
        nc.vector.memset(neg87_t[:], -87.0)

        # --- phase A+B: per half-T: stream x once, project, softplus, stage
        # dl/du to DRAM. x is streamed through transient buffers for the
        # projection and re-loaded from DRAM for the du multiply, so the two
        # halves' projections are not serialized by x-tile residency.
        bb_ts, cb_ts = [], []

        def emit_a_head(h):
            hsl = slice(h * TH, (h + 1) * TH)
            xdbl_p = pall.tile([128, TH], F, tag="work", name=f"xdbl_{h}")
            dma_engs = (nc.sync, nc.gpsimd)
            for i in range(NT):
                xt = xpool.tile([128, TH], H16, tag="xs", bufs=4)
                dma_engs[i % 2].dma_start(xt[:], x_d[i * 128:(i + 1) * 128, hsl])
                if h == 0:
                    wt = wpool.tile([128, C44], H16, tag="w", bufs=NT)
                    nc.scalar.dma_start(wt[:], wxT_d[i * 128:(i + 1) * 128, :])
                    wts.append(wt)
                for jj in range(TH // 512):
                    c0 = jj * 512
                    nc.tensor.matmul(
                        xdbl_p[0:C44, c0:c0 + 512], wts[i][:],
                        xt[:, jj * 512:(jj + 1) * 512],
                        start=(i == 0), stop=(i == NT - 1))
            nc.vector.tensor_copy(dtr_t[:, hsl], xdbl_p[0:R, 0:TH])
            nc.vector.tensor_copy(bc_t[:, hsl], xdbl_p[32:32 + N, 0:TH])
            nc.vector.tensor_copy(cc_t[:, hsl], xdbl_p[64:64 + N, 0:TH])
            nc.sync.dma_start(bc_dram[:, hsl], bc_t[:, hsl])
            nc.sync.dma_start(cc_dram[:, hsl], cc_t[:, hsl])
            # broadcast B/C into scan layout for this half as soon as the
            # half's bc/cc rows are staged (keeps SP ahead of phase C)
            for gp in range(NG // 2):
                for nm, src_dram, lst in (("b", bc_dram, bb_ts), ("c", cc_dram, cb_ts)):
                    if h == 0:
                        bt = singles.tile([128, 2 * T], H16, tag=f"bcast_{nm}{gp}",
                                          name=f"bcast_{nm}{gp}")
                        lst.append(bt)
                    bt = lst[gp]
                    for c in range(2):
                        src = bass.AP(tensor=src_dram[:, :].tensor,
                                      offset=(gp * 2 + c) * 4 * T + h * TH,
                                      ap=[[0, 32], [T, 4], [1, TH]])
                        nc.scalar.dma_start(
                            bt[:, c * T + h * TH:c * T + (h + 1) * TH], src)

        def emit_a_i(h, i):
            hsl = slice(h * TH, (h + 1) * TH)
            # dts = Wdt @ dtr  (contraction R=12), half-T chunk
            dts_p = pall.tile([128, TH], F, tag="work")
            for jj in range(TH // 512):
                nc.tensor.matmul(
                    dts_p[:, jj * 512:(jj + 1) * 512],
                    wdtT_t[:, i * 128:(i + 1) * 128],
                    dtr_t[:, h * TH + jj * 512:h * TH + (jj + 1) * 512],
                    start=True, stop=True)
            # delta = softplus(dts + bias) = ln(exp(dts + bias) + 1) -> bf16
            de_t = dpool.tile([128, TH], F, tag="de")
            nc.scalar.activation(de_t[:], dts_p[:], AF.Exp,
                                 bias=bias_t[:, i:i + 1])
            dl_t = dpool.tile([128, TH], H16, tag="dl")
            nc.scalar.activation(dl_t[:], de_t[:], AF.Ln, bias=1.0)
            nc.sync.dma_start(dl_drams[i][:, hsl], dl_t[:])
            # du = delta * u -> bf16 -> DRAM (x re-loaded from DRAM)
            xr_t = xpool.tile([128, TH], H16, tag="xr", bufs=3)
            nc.sync.dma_start(xr_t[:], x_d[i * 128:(i + 1) * 128, hsl])
            du_t = dpool.tile([128, TH], H16, tag="du")
            nc.gpsimd.tensor_mul(du_t[:], dl_t[:], xr_t[:])
            nc.sync.dma_start(du_drams[i][:, hsl], du_t[:])

        pending_ys = []

        def flush_ys():
            while pending_ys:
                pi, py_hs = pending_ys.pop(0)
                ys_t = ypool.tile([128, T], H16, tag="ys", bufs=1)
                for hh in range(2):
                    nc.scalar.copy(ys_t[:, hh * TH:(hh + 1) * TH], py_hs[hh][:])
                nc.sync.dma_start(ys_d[pi * 128:(pi + 1) * 128, :], ys_t[:])

        def emit_c(i):
            # per d-tile scan: full-T sub-block scans accumulating y in two
            # half-T PSUM tiles (so the next tile's PE work overlaps drain)
            y_hs = [pall.tile([128, TH], F, tag="y", name=f"y_{i}_{hh}")
                    for hh in range(2)]
            for s in range(4):
                i2 = i * 4 + s
                # delta/du broadcast into scan layout via fp16 DRAM reload
                dlb_t = spool.tile([128, T], H16, tag="dlb", bufs=4)
                dub_t = spool.tile([128, T], H16, tag="dub", bufs=4)
                for dr, dst in ((dl_drams[i], dlb_t), (du_drams[i], dub_t)):
                    src = bass.AP(tensor=dr[:, :].tensor,
                                  offset=s * 32 * T,
                                  ap=[[T, 32], [0, 4], [1, T]])
                    nc.sync.dma_start(dst[:], src)
                dub3 = bass.AP(tensor=dub_t[:].tensor, offset=dub_t[:].offset,
                               ap=[dub_t[:].ap[0], [0, 2], [1, T]])
                for gp in range(NG // 2):
                    b16_t = spool.tile([128, 2 * T], H16, tag="b16", bufs=3)
                    h16_t = spool.tile([128, 2 * T], H16, tag="h16", bufs=3)
                    hc_t = spool.tile([128, 2 * T], H16, tag="hc", bufs=2)
                    b163 = bass.AP(tensor=b16_t[:].tensor,
                                   offset=b16_t[:].offset,
                                   ap=[b16_t[:].ap[0], [T, 2], [1, T]])
                    bb3 = bass.AP(tensor=bb_ts[gp][:].tensor,
                                  offset=bb_ts[gp][:].offset,
                                  ap=[bb_ts[gp][:].ap[0], [T, 2], [1, T]])
                    nc.gpsimd.tensor_mul(b163, dub3, bb3)
                    # one fused scan over the c-pair [c0 | c1]: the first dA
                    # column of the c1 half is exp(-87) ~= 0, which resets the
                    # running state to bu[0] exactly as a fresh 0-init scan
                    da2_t = spool.tile([128, 2 * T], H16, tag="da", bufs=3)
                    g0, g1 = gp * 2, gp * 2 + 1
                    nc.scalar.activation(
                        da2_t[:, 0:T], dlb_t[:], AF.Exp,
                        scale=a_t[:, i2 * 4 + g0:i2 * 4 + g0 + 1])
                    nc.scalar.activation(
                        da2_t[:, T:T + 1], dlb_t[:, 0:1], AF.Exp,
                        scale=0.0, bias=neg87_t[:])
                    nc.scalar.activation(
                        da2_t[:, T + 1:2 * T], dlb_t[:, 1:T], AF.Exp,
                        scale=a_t[:, i2 * 4 + g1:i2 * 4 + g1 + 1])
                    nc.vector.tensor_tensor_scan(
                        h16_t[:], da2_t[:], b16_t[:],
                        0.0, op0=OP.mult, op1=OP.add)
                    cb3 = bass.AP(tensor=cb_ts[gp][:].tensor,
                                  offset=cb_ts[gp][:].offset,
                                  ap=[[cb_ts[gp][:].ap[0][0], 128], [T, 2], [1, T]])
                    hc3 = bass.AP(tensor=hc_t[:].tensor,
                                  offset=hc_t[:].offset,
                                  ap=[hc_t[:].ap[0], [T, 2], [1, T]])
                    h163 = bass.AP(tensor=h16_t[:].tensor,
                                   offset=h16_t[:].offset,
                                   ap=[h16_t[:].ap[0], [T, 2], [1, T]])
                    hc_eng = nc.gpsimd if gp == 0 else nc.vector
                    hc_eng.tensor_mul(hc3, h163, cb3)
                    for c in range(2):
                        g = gp * 2 + c
                        for jj in range(T // 512):
                            nc.tensor.matmul(
                                y_hs[jj // 2][:, (jj % 2) * 512:(jj % 2 + 1) * 512],
                                s4_t[:, s * 128:(s + 1) * 128],
                                hc_t[:, c * T + jj * 512:c * T + (jj + 1) * 512],
                                start=(s == 0 and g == 0),
                                stop=(s == 3 and g == NG - 1))
                if s == 0:
                    # drain the previous tile's y after this tile's first dA
                    # exps are queued, so Act never starves the scan chain
                    flush_ys()
            pending_ys.append((i, y_hs))

        emit_a_head(0)
        emit_a_head(1)
        for i in range(NT):
            emit_a_i(0, i)
            emit_a_i(1, i)
            if i >= 1:
                emit_c(i - 1)
        emit_c(NT - 1)
        flush_ys()
    nc.compile()
    return nc


def _build_merge_nc(affine=True):
    nc = bacc.Bacc()
    ys0_d = nc.dram_tensor("ys0", [D, TH], H16, kind="ExternalInput")
    ys1_d = nc.dram_tensor("ys1", [D, TH], H16, kind="ExternalInput")
    u_d = nc.dram_tensor("u", [D, TH], H16, kind="ExternalInput")
    ds_d = nc.dram_tensor("ds_r", [128, 2 * NT], F, kind="ExternalInput")
    lnw_d = nc.dram_tensor("lnw", [1, D_INNER], F, kind="ExternalInput")
    lnb_d = nc.dram_tensor("lnb", [1, D_INNER], F, kind="ExternalInput")
    out_d = nc.dram_tensor("out", [TH, J, D_INNER], H16, kind="ExternalOutput")

    NJ = TH // 128  # 8 t-tiles

    with tile.TileContext(nc) as tc, ExitStack() as ctx:
        singles = ctx.enter_context(tc.tile_pool(name="singles", bufs=1))
        mpool = ctx.enter_context(tc.tile_pool(name="mpool", bufs=3))
        lpool = ctx.enter_context(tc.tile_pool(name="lpool", bufs=8))
        opool = ctx.enter_context(tc.tile_pool(name="opool", bufs=5))
        ptr = ctx.enter_context(tc.tile_pool(name="ptr", bufs=4, space="PSUM"))

        ds_t = singles.tile([128, 2 * NT], F)
        nc.sync.dma_start(ds_t[:], ds_d[:, :])
        dsum_t = singles.tile([128, NT], F)
        nc.vector.tensor_add(dsum_t[:], ds_t[:, 0:NT], ds_t[:, NT:2 * NT])
        if affine:
            lnw_t = singles.tile([128, D_INNER], F)
            nc.sync.dma_start(lnw_t[:], bass.AP(
                tensor=lnw_d[:, :].tensor, offset=0, ap=[[0, 128], [1, D_INNER]]))
            lnb_t = singles.tile([128, D_INNER], F)
            nc.sync.dma_start(lnb_t[:], bass.AP(
                tensor=lnb_d[:, :].tensor, offset=0, ap=[[0, 128], [1, D_INNER]]))
        else:
            # weights are ones/zeros: keep the inputs referenced via a tiny load
            lnw_t = singles.tile([1, D_INNER], F)
            nc.sync.dma_start(lnw_t[:], lnw_d[:, :])
            lnb_t = singles.tile([1, D_INNER], F)
            nc.sync.dma_start(lnb_t[:], lnb_d[:, :])
        eps_t = singles.tile([128, 1], F)
        nc.vector.memset(eps_t[:], 1e-5)
        ident = singles.tile([128, 128], H16)
        make_identity(nc, ident[:])

        # phase 1+2a: ym[i] = ys0 + ys1 + Dsum*u, transposed into one resident
        # t-major buffer (jj-major blocks) as each d-tile completes. Four
        # transposes share a PSUM tile so one wide strided copy drains them.
        ymT_all = singles.tile([128, NJ * D], H16, name="ymT_all")
        for i in range(NT):
            y0 = mpool.tile([128, TH], H16, tag="y0")
            nc.sync.dma_start(y0[:], ys0_d[i * 128:(i + 1) * 128, :])
            y1 = mpool.tile([128, TH], H16, tag="y1")
            nc.gpsimd.dma_start(y1[:], ys1_d[i * 128:(i + 1) * 128, :])
            ut = mpool.tile([128, TH], H16, tag="ut")
            nc.sync.dma_start(ut[:], u_d[i * 128:(i + 1) * 128, :])
            ym0 = mpool.tile([128, TH], H16, tag="ym0")
            nc.vector.tensor_add(ym0[:], y0[:], y1[:])
            # ym = ym0 + Dsum * u  (TSP mult runs in the 4x DVE mode)
            ym1 = mpool.tile([128, TH], H16, tag="ym1")
            nc.vector.tensor_scalar(ym1[:], ut[:], dsum_t[:, i:i + 1], None,
                                    op0=OP.mult)
            ym = mpool.tile([128, TH], H16, tag="ym")
            nc.gpsimd.tensor_add(ym[:], ym0[:], ym1[:])
            for half in range(2):
                trp4 = ptr.tile([128, 512], H16, tag="trp")
                for q in range(4):
                    jj = half * 4 + q
                    nc.tensor.transpose(trp4[:, q * 128:(q + 1) * 128],
                                        ym[:, jj * 128:(jj + 1) * 128], ident[:])
                dst = bass.AP(tensor=ymT_all[:].tensor,
                              offset=ymT_all[:].offset + (half * 4) * D + i * 128,
                              ap=[ymT_all[:].ap[0], [D, 4], [1, 128]])
                src = bass.AP(tensor=trp4[:].tensor, offset=trp4[:].offset,
                              ap=[trp4[:].ap[0], [128, 4], [1, 128]])
                if (i + half) % 2:
                    nc.scalar.copy(dst, src)
                else:
                    nc.vector.tensor_copy(dst, src)

        # phase 2b: per t-block LayerNorm, stages batched across joints
        for jj in range(NJ):
            base = ymT_all[:].offset + jj * D
            yvs = [bass.AP(tensor=ymT_all[:].tensor, offset=base + h,
                           ap=[ymT_all[:].ap[0], [J, D_INNER]]) for h in range(J)]
            mvs, rsds, nbs = [], [], []
            for h in range(J):
                stats = lpool.tile([128, 6], F, tag="stats")
                nc.vector.bn_stats(out=stats[:], in_=yvs[h])
                mv = lpool.tile([128, 2], F, tag="mv")
                nc.vector.bn_aggr(out=mv[:], in_=stats[:])
                mvs.append(mv)
            sds = []
            for h in range(J):
                sd = lpool.tile([128, 1], F, tag="sd")
                nc.scalar.activation(sd[:], mvs[h][:, 1:2], AF.Sqrt, bias=eps_t[:])
                sds.append(sd)
            for h in range(J):
                rsd = lpool.tile([128, 1], F, tag="rsd")
                nc.vector.reciprocal(rsd[:], sds[h][:])
                rsds.append(rsd)
                nb = lpool.tile([128, 1], F, tag="nb")
                nc.vector.tensor_scalar(nb[:], mvs[h][:, 0:1], rsd[:], -1.0,
                                        op0=OP.mult, op1=OP.mult)
                nbs.append(nb)
            for h in range(J):
                # nrm = (yv - mu) * rstd: Identity(yv*rstd + (-mu*rstd))
                nrm = opool.tile([128, D_INNER], H16, tag="nrm")
                nc.scalar.activation(nrm[:], yvs[h], AF.Identity,
                                     bias=nbs[h][:], scale=rsds[h][:])
                if affine:
                    o1 = opool.tile([128, D_INNER], H16, tag="o1")
                    nc.vector.tensor_mul(o1[:], nrm[:], lnw_t[:])
                    o2 = opool.tile([128, D_INNER], H16, tag="o2")
                    nc.vector.tensor_add(o2[:], o1[:], lnb_t[:])
                else:
                    o2 = nrm
                dst = bass.AP(tensor=out_d[:, :, :].tensor,
                              offset=jj * 128 * J * D_INNER + h * D_INNER,
                              ap=[[J * D_INNER, 128], [1, D_INNER]])
                (nc.scalar if (jj * J + h) % 2 else nc.sync).dma_start(dst, o2[:])
    nc.compile()
    return nc


_CACHE = {}


def _get_ncs(affine=True):
    if "scan" not in _CACHE:
        _CACHE["scan"] = _build_scan_nc()
    mk = f"merge_{affine}"
    if mk not in _CACHE:
        _CACHE[mk] = _build_merge_nc(affine)
    return _CACHE["scan"], _CACHE[mk]


def kernel(x, x_proj_weight, dt_projs_weight, dt_projs_bias, A_logs, Ds,
           ln_weight, ln_bias):
    x = np.asarray(x, np.float32)
    affine = not (np.all(np.asarray(ln_weight) == 1.0)
                  and np.all(np.asarray(ln_bias) == 0.0))
    nc_scan, nc_merge = _get_ncs(affine)

    # host-side sharding prep (reshapes/transposes only)
    xflat = np.ascontiguousarray(
        np.transpose(np.asarray(x), (0, 1, 3, 2)).reshape(B, D, T))
    xflat16 = xflat.astype(np.float16)

    # scan-layout constant reshapes, per direction k
    p = np.arange(128)
    i2 = np.arange(NS)
    g = np.arange(NG)
    # alog_r[p, i2*NG+g] = A_logs[k*D + i2*32 + p//4, g*4 + p%4]
    d_idx = (i2[None, :, None] * 32 + (p // 4)[:, None, None])          # (128,NS,1)
    n_idx = (g[None, None, :] * 4 + (p % 4)[:, None, None])             # (128,1,NG) -> bc
    q = np.arange(128)
    pp = np.arange(128)
    wx = np.asarray(x_proj_weight, np.float32)
    wxT_pad = np.zeros((K, D, 80), np.float32)
    for k in range(K):
        wxT_pad[k, :, 0:R] = wx[k, 0:R].T
        wxT_pad[k, :, 32:32 + N] = wx[k, R:R + N].T
        wxT_pad[k, :, 64:64 + N] = wx[k, R + N:R + 2 * N].T
    s4 = np.zeros((128, 512), np.float32)
    for sb in range(4):
        # y_all[p] += hc[q] where p = 32*sb + q//4
        s4[:, sb * 128:(sb + 1) * 128] = (pp[None, :] == (sb * 32 + q // 4)[:, None])
    s4 = s4.astype(np.float16)

    in_maps = []
    for c in range(8):
        b, k = c // 2, c % 2
        xk = xflat16[b] if k == 0 else xflat16[b][:, ::-1]
        al = np.asarray(A_logs, np.float32)[k * D:(k + 1) * D]          # (D, N)
        alog_r = al[d_idx, n_idx]                                        # (128, NS, NG)
        in_maps.append(dict(
            x=np.ascontiguousarray(xk),
            wxT=np.ascontiguousarray(wxT_pad[k]).astype(np.float16),
            wdtT=np.ascontiguousarray(np.asarray(dt_projs_weight, np.float32)[k].T).astype(np.float16),
            bias_r=np.ascontiguousarray(
                np.asarray(dt_projs_bias, np.float32)[k].reshape(NT, 128).T),
            alog_r=np.ascontiguousarray(alog_r.reshape(128, NS * NG)),
            s4=s4,
        ))
    res1 = run_bass_kernel_spmd(nc_scan, in_maps, core_ids=list(range(8))).results

    ds_np = np.asarray(Ds, np.float32)
    ds_r = np.concatenate(
        [ds_np[k * D:(k + 1) * D].reshape(NT, 128).T for k in range(K)], axis=1)
    lnw = np.asarray(ln_weight, np.float32).reshape(1, D_INNER)
    lnb = np.asarray(ln_bias, np.float32).reshape(1, D_INNER)

    in_maps2 = []
    for c in range(8):
        b, th = c // 2, c % 2
        sl = slice(th * TH, (th + 1) * TH)
        ys0 = res1[2 * b]["ys"][:, sl]
        ys1 = res1[2 * b + 1]["ys"][:, ::-1][:, sl]
        in_maps2.append(dict(
            ys0=np.ascontiguousarray(ys0),
            ys1=np.ascontiguousarray(ys1),
            u=np.ascontiguousarray(xflat16[b][:, sl]),
            ds_r=np.ascontiguousarray(ds_r),
            lnw=lnw, lnb=lnb,
        ))
    res2 = run_bass_kernel_spmd(nc_merge, in_maps2, core_ids=list(range(8))).results

    out = np.empty((B, T, J, D_INNER), np.float32)
    for c in range(8):
        b, th = c // 2, c % 2
        out[b, th * TH:(th + 1) * TH] = res2[c]["out"].astype(np.float32)
    return out

